# revision 20
# baseline (speedup 1.0000x reference)
"""Trainium2 Bass kernel: batched multi-head self-attention (nn_Attention).

y = softmax(q k^T / sqrt(64)) v, projected; x (8, 1025, 768), 12 heads x 64.

Strategy: batch-parallel across the 8 NeuronCores (one batch element per
core, no collectives). Per core, everything is feature-major (transposed).

Software-pipelined single pass: the qk projection for head-pair p+1, the
v projection, and the output projection are woven as filler matmuls between
the attention steps of pair p, so the scalar engine (softmax exp) starts
early and never stalls behind a projection phase. Score matmuls for the two
heads of a pair run CONCURRENTLY in the PE array (64x128 row tiling: h0 in
rows 0-63, h1 in rows 64-127, distinct PSUM banks). Scores for both heads
of one key tile land in one [128, 1024] PSUM tile so each key tile needs a
single 1024-wide exp. The 8-key tail tile puts h0 at cols 0:512 and h1 at
512:1024 (rows 0:8 both) so one exp covers it and no v-row duplication is
needed. Softmax normalization runs per-pair right after each pair's block,
entirely off the PE queue (DVE reciprocal, Pool-engine partition
broadcast, DVE multiplies) so it never stalls matmuls; the kernel tail
only waits on pair 5's short chain before the last output projections. Inputs stream in as
full-width tiles (2KB rows) interleaved so the first projection starts as
early as the DMA allows; wqk is column-grouped by head-pair on the host so
the first pair's weights are one contiguous slice.

Operands fp16 (inputs/weights/q/k/v), exp'd weights bf16, accumulation
fp32 in PSUM.
"""
import sys

try:
    import concourse.bass  # noqa: F401
except ImportError:
    sys.path.insert(0, "/opt/trn_rl_repo")

import numpy as np

from contextlib import ExitStack

import concourse.bass as bass
import concourse.tile as tile
from concourse import bacc, mybir

F32 = mybir.dt.float32
BF16 = mybir.dt.bfloat16
F16 = mybir.dt.float16

C = 768
H = 12
D = 64
NTOK = 1025
T = 1032
CT = C // 128
SCALE = D ** -0.5

KT = [(i * 128, 128) for i in range(8)] + [(1024, 8)]
QC = [(0, 512), (512, 512)]
QC2 = (1024, 8)
VC = [(0, 512), (512, 256)]
VW = 65


def build(matmul_dtype="fp16"):
    MT = AT = F16
    ATTN = BF16
    nc = bacc.Bacc("TRN2", target_bir_lowering=False, debug=False, num_devices=8)

    xT_d = nc.dram_tensor("xT", [C, T], MT, kind="ExternalInput")
    # wqkT columns grouped by pair: [q_p0 | k_p0 | q_p1 | k_p1 | ...] (256 each)
    wqkT_d = nc.dram_tensor("wqkT", [C, 2 * C], MT, kind="ExternalInput")
    wvT_d = nc.dram_tensor("wvT", [C, C], MT, kind="ExternalInput")
    wpT_d = nc.dram_tensor("wpT", [C, C], MT, kind="ExternalInput")
    bp_d = nc.dram_tensor("bp", [C, 1], F32, kind="ExternalInput")
    yT_d = nc.dram_tensor("yT", [C, T], F16, kind="ExternalOutput")

    with tile.TileContext(nc) as tc, ExitStack() as ctx:
        p_x = ctx.enter_context(tc.tile_pool(name="x", bufs=1))
        p_w = ctx.enter_context(tc.tile_pool(name="w", bufs=1))
        p_qk = ctx.enter_context(tc.tile_pool(name="qk", bufs=1))
        p_v = ctx.enter_context(tc.tile_pool(name="v", bufs=1))
        p_ao = ctx.enter_context(tc.tile_pool(name="ao", bufs=1))
        p_at = ctx.enter_context(tc.tile_pool(name="at", bufs=1))
        p_sm = ctx.enter_context(tc.tile_pool(name="sm", bufs=1))
        p_stage = ctx.enter_context(tc.tile_pool(name="stage", bufs=1))
        ps_sc = ctx.enter_context(tc.tile_pool(name="psc", bufs=2, space="PSUM"))
        ps_av = ctx.enter_context(tc.tile_pool(name="psav", bufs=2, space="PSUM"))
        ps_pr = ctx.enter_context(tc.tile_pool(name="pspr", bufs=2, space="PSUM"))

        xT = [p_x.tile([128, T], MT, tag=f"x{i}", name=f"x{i}") for i in range(CT)]
        wqk = [p_w.tile([128, 2 * C], MT, tag=f"wqk{i}", name=f"wqk{i}")
               for i in range(CT)]
        wv = [p_w.tile([128, C], MT, tag=f"wv{i}", name=f"wv{i}") for i in range(CT)]
        wp = [p_w.tile([128, C], MT, tag=f"wp{i}", name=f"wp{i}") for i in range(CT)]
        bp_sb = [p_w.tile([128, 1], F32, tag=f"bp{i}", name=f"bp{i}")
                 for i in range(CT)]
        qkT = [p_qk.tile([128, T], AT, tag=f"qkT{i}", name=f"qkT{i}")
               for i in range(12)]
        v_ext = [p_v.tile([128, H * VW + 63], AT, tag=f"v{i}", name=f"v{i}")
                 for i in range(9)]
        aoT = [p_ao.tile([128, T], MT, tag=f"ao{i}", name=f"ao{i}")
               for i in range(CT)]

        # full-width tiles (2KB rows — small-packet DMA runs at a fraction of
        # peak), interleaved so pair 0's weights arrive alongside x
        for c in range(CT):
            nc.sync.dma_start(xT[c][:], xT_d.ap()[c * 128:(c + 1) * 128, :])
            nc.sync.dma_start(wqk[c][:, 0:256],
                              wqkT_d.ap()[c * 128:(c + 1) * 128, 0:256])
        for c in range(CT):
            nc.sync.dma_start(wv[c][:], wvT_d.ap()[c * 128:(c + 1) * 128, :])
        for c in range(CT):
            nc.sync.dma_start(wqk[c][:, 256:1536],
                              wqkT_d.ap()[c * 128:(c + 1) * 128, 256:1536])
        for c in range(CT):
            nc.sync.dma_start(wp[c][:], wpT_d.ap()[c * 128:(c + 1) * 128, :])
            nc.sync.dma_start(bp_sb[c][:], bp_d.ap()[c * 128:(c + 1) * 128, :])

        def wqk_col(p, is_k):
            return 256 * p + 128 * is_k

        # ---------- filler units (each emits ~one PSUM chunk of proj work) ----

        def qk_unit(p, is_k, qoff, qsz):
            ot = 6 * is_k + p

            def go():
                ps = ps_pr.tile([128, 512], F32, tag="pr", name="ps_pr")
                for c in range(CT):
                    nc.tensor.matmul(
                        ps[:, :qsz],
                        wqk[c][:, wqk_col(p, is_k):wqk_col(p, is_k) + 128],
                        xT[c][:, qoff:qoff + qsz],
                        start=(c == 0), stop=(c == CT - 1),
                    )
                nc.vector.tensor_copy(qkT[ot][:, qoff:qoff + qsz], ps[:, :qsz])
            return go

        def qk_pair_units(p):
            order = [(p, 0, QC[0]), (p, 1, QC[0]), (p, 1, QC[1]),
                     (p, 1, QC2), (p, 0, QC[1]), (p, 0, QC2)]
            return [qk_unit(pp, k, qoff, qsz) for pp, k, (qoff, qsz) in order]

        def v_unit(nt, voff, vsz):
            noff, nsz = KT[nt]

            def go():
                ps = ps_pr.tile([128, 512], F32, tag="pr", name="ps_pr")
                for c in range(CT):
                    nc.tensor.matmul(
                        ps[:nsz, :vsz],
                        xT[c][:, noff:noff + nsz],
                        wv[c][:, voff:voff + vsz],
                        start=(c == 0), stop=(c == CT - 1),
                    )
                nh = vsz // D
                h0 = voff // D
                dst = (
                    v_ext[nt][0:nsz, h0 * VW:(h0 + nh) * VW]
                    .rearrange("p (hh w) -> p hh w", w=VW)[:, :, 0:D]
                )
                src = ps[0:nsz, 0:vsz].rearrange("p (hh w) -> p hh w", w=D)
                nc.vector.tensor_copy(dst, src)
                if voff + vsz == C:
                    # ones column (valid tokens only) + zeroed pad/tail
                    if nt < 8:
                        ones_col = (
                            v_ext[nt][0:nsz, 0:H * VW]
                            .rearrange("p (hh w) -> p hh w", w=VW)[:, :, D:VW]
                        )
                        _memset(nc, AT, ones_col, one=True)
                    else:
                        pad_col = (
                            v_ext[nt][0:nsz, 0:H * VW]
                            .rearrange("p (hh w) -> p hh w", w=VW)[:, :, D:VW]
                        )
                        _memset(nc, AT, pad_col, one=False)
                        one_row = (
                            v_ext[nt][0:1, 0:H * VW]
                            .rearrange("p (hh w) -> p hh w", w=VW)[:, :, D:VW]
                        )
                        _memset(nc, AT, one_row, one=True)
                    _memset(nc, AT, v_ext[nt][:, H * VW:H * VW + 63], one=False)
            return go

        def v_units():
            return [v_unit(nt, voff, vsz) for nt in range(9) for (voff, vsz) in VC]

        def e_unit(ot, qoff, qsz):
            def go():
                ps = ps_pr.tile([128, 512], F32, tag="pr", name="ps_pr")
                for c in range(CT):
                    nc.tensor.matmul(
                        ps[:, :qsz],
                        wp[c][:, ot * 128:(ot + 1) * 128],
                        aoT[c][:, qoff:qoff + qsz],
                        start=(c == 0), stop=(c == CT - 1),
                    )
                st = p_stage.tile([128, 512], F16, tag="ystage", name="ystage",
                                  bufs=4)
                nc.vector.tensor_scalar_add(st[:, :qsz], ps[:, :qsz],
                                            bp_sb[ot][:, 0:1])
                nc.sync.dma_start(
                    yT_d.ap()[ot * 128:(ot + 1) * 128, qoff:qoff + qsz],
                    st[:, :qsz])
            return go

        # ---------- attention block for one head pair, one query chunk -------

        def normalize(av_ps, pair, h_in_pair, qoff, qsz):
            # single-head normalize (tail queries: recip of [1, 8] is cheap)
            p0 = 64 * h_in_pair
            av_sb = p_sm.tile([VW, 512], F32, tag="avsb", name="avsb", bufs=2)
            nc.vector.tensor_copy(av_sb[0:VW, 0:qsz], av_ps[0:VW, 0:qsz])
            rec = p_sm.tile([1, 512], F32, tag="rec", name="rec", bufs=2)
            nc.vector.reciprocal(rec[0:1, 0:qsz], av_sb[D:VW, 0:qsz])
            bc = p_sm.tile([64, 512], F32, tag="bc", name="bc", bufs=2)
            nc.gpsimd.partition_broadcast(bc[0:64, 0:qsz], rec[0:1, 0:qsz])
            nc.vector.tensor_mul(
                aoT[pair][p0:p0 + 64, qoff:qoff + qsz],
                av_sb[0:D, 0:qsz],
                bc[0:64, 0:qsz],
            )

        # big query chunks: stash all 12 heads' av into one wide tile so the
        # AV psum banks free quickly
        def make_chunk_state():
            return {"av": p_sm.tile([VW, 12 * 512], F32, tag="avall",
                                    name="avall", bufs=2)}

        def stash_av(cs, av_ps, h, qoff, qsz):
            nc.vector.tensor_copy(cs["av"][0:VW, h * 512:h * 512 + qsz],
                                  av_ps[0:VW, 0:qsz])

        # per-pair normalize, fully off the PE queue and cheap on the DVE.
        # DVE reciprocal costs ~7ns/elem PER LANE (lanes = partitions), so a
        # tiny SBUF->SBUF DMA first spreads the pair's 1024 denominators
        # across 64 partitions ([64,16] -> ~0.4us reciprocal), a second DMA
        # flattens the result back to one partition for the Pool-engine
        # partition-broadcast (which only accepts partition-0 sources), and
        # two DVE multiplies write aoT. The three stages are emitted a block
        # apart so every op is ready the moment its strict-FIFO engine queue
        # dequeues it.
        def pair_norm_a(cs, pair):
            den64 = p_sm.tile([64, 16], F32, tag="den64", name="den64", bufs=3)
            nc.sync.dma_start(
                den64[0:64, 0:16],
                cs["av"][D:VW, 2 * pair * 512:(2 * pair + 2) * 512])
            return den64

        def pair_norm_b(cs, pair, den64, qsz):
            rec64 = p_sm.tile([64, 16], F32, tag="rec64", name="rec64", bufs=3)
            nc.vector.reciprocal(rec64[0:64, 0:16], den64[0:64, 0:16])
            rec = p_sm.tile([1, 1024], F32, tag="recp", name="recp", bufs=3)
            nc.sync.dma_start(rec[0:1, 0:1024], rec64[0:64, 0:16])
            bcs = []
            for hi in range(2):
                bc = p_sm.tile([64, 512], F32, tag="bcp", name="bcp", bufs=6)
                nc.gpsimd.partition_broadcast(
                    bc[0:64, 0:qsz], rec[0:1, hi * 512:hi * 512 + qsz])
                bcs.append(bc)
            return bcs

        def pair_norm_c(cs, pair, bcs, qoff, qsz):
            for hi in range(2):
                h = 2 * pair + hi
                nc.vector.tensor_mul(
                    aoT[pair][64 * hi:64 * hi + 64, qoff:qoff + qsz],
                    cs["av"][0:D, h * 512:h * 512 + qsz],
                    bcs[hi][0:64, 0:qsz],
                )

        def attn_block(pair, qoff, qsz, fillers, fill_per_kt, cs, warm=False):
            h0, h1 = 2 * pair, 2 * pair + 1
            fill_iter = iter(fillers)

            def fill(n):
                for _ in range(n):
                    f = next(fill_iter, None)
                    if f is not None:
                        f()

            avs = {
                h0: ps_av.tile([128, 512], F32, tag="av", name="ps_av"),
                h1: ps_av.tile([128, 512], F32, tag="av", name="ps_av"),
            }
            pend = []  # (kt, at_tile) awaiting AV emission

            def scores_kt(kt, warm=False):
                koff, ksz = KT[kt]
                sc = ps_sc.tile([128, 1024], F32, tag="sc", name="ps_sc")
                if warm:
                    # full-array dummy matmul holds the PE clock governor at
                    # 8/8 through the half-array score runs (overwritten by
                    # the real scores below)
                    nc.tensor.matmul(
                        sc[0:128, 0:512],
                        qkT[0][0:128, 0:128],
                        qkT[0][0:128, 0:512],
                        start=True, stop=True,
                    )
                # the two heads' score matmuls run concurrently (row groups
                # 0/64) into one psum tile: h0 at cols 0:512, h1 at 512:1024
                nc.tensor.matmul(
                    sc[0:ksz, 0:qsz],
                    qkT[6 + pair][0:64, koff:koff + ksz],
                    qkT[pair][0:64, qoff:qoff + qsz],
                    start=True, stop=True,
                )
                nc.tensor.matmul(
                    sc[0:ksz, 512:512 + qsz],
                    qkT[6 + pair][64:128, koff:koff + ksz],
                    qkT[pair][64:128, qoff:qoff + qsz],
                    start=True, stop=True,
                )
                at = p_at.tile([128, 1024], ATTN, tag="attnT", name="attnT", bufs=6)
                nc.scalar.activation(
                    at[0:ksz, 0:1024], sc[0:ksz, 0:1024],
                    mybir.ActivationFunctionType.Exp, scale=SCALE,
                )
                return (kt, at)

            def drain_av():
                while pend:
                    pkt, pat = pend.pop(0)
                    _, pksz = KT[pkt]
                    for hi, h in enumerate((h0, h1)):
                        nc.tensor.matmul(
                            avs[h][0:128, 0:qsz],
                            v_ext[pkt][0:pksz, h * VW:h * VW + 128],
                            pat[0:pksz, hi * 512:hi * 512 + qsz],
                            start=(pkt == 0), stop=False,
                            skip_group_check=True,
                        )

            # 2-kt bursts keep the PE in 64-row tiling mode across 4 score
            # matmuls before switching to 128-row AV/filler matmuls; AV
            # emission lags one burst so its exp input is ready
            for kb in range(4):
                new = [scores_kt(2 * kb, warm=warm), scores_kt(2 * kb + 1)]
                fill(fill_per_kt)
                drain_av()
                pend.extend(new)
                fill(fill_per_kt)

            # tail key tile (8 keys, rows 0:8): h0 scores at cols 0:512
            # (bank 0), h1 at 512:1024 (bank 1) -> one [8, 1024] exp, and the
            # tail AV reads v_ext[8] rows 0:8 for both heads
            koff, ksz = KT[8]
            sc8 = ps_sc.tile([128, 1024], F32, tag="sc", name="ps_sc")
            nc.tensor.matmul(
                sc8[0:8, 0:qsz],
                qkT[6 + pair][0:64, koff:koff + ksz],
                qkT[pair][0:64, qoff:qoff + qsz],
                start=True, stop=True,
            )
            nc.tensor.matmul(
                sc8[0:8, 512:512 + qsz],
                qkT[6 + pair][64:128, koff:koff + ksz],
                qkT[pair][64:128, qoff:qoff + qsz],
                start=True, stop=True,
            )
            at8 = p_at.tile([128, 1024], ATTN, tag="attnT", name="attnT", bufs=6)
            nc.scalar.activation(
                at8[0:8, 0:1024], sc8[0:8, 0:1024],
                mybir.ActivationFunctionType.Exp, scale=SCALE,
            )
            # drain pending AV (kt 6/7), then the tail AV closes accumulation
            while pend:
                pkt, pat = pend.pop(0)
                _, pksz = KT[pkt]
                for hi, h in enumerate((h0, h1)):
                    nc.tensor.matmul(
                        avs[h][0:128, 0:qsz],
                        v_ext[pkt][0:pksz, h * VW:h * VW + 128],
                        pat[0:pksz, hi * 512:hi * 512 + qsz],
                        start=False, stop=False,
                        skip_group_check=True,
                    )
            for hi, h in enumerate((h0, h1)):
                nc.tensor.matmul(
                    avs[h][0:128, 0:qsz],
                    v_ext[8][0:8, h * VW:h * VW + 128],
                    at8[0:8, hi * 512:hi * 512 + qsz],
                    start=False, stop=True,
                    skip_group_check=True,
                )
            stash_av(cs, avs[h0], h0, qoff, qsz)
            stash_av(cs, avs[h1], h1, qoff, qsz)
            fill(99)  # drain leftovers

        # ---------- tail-query (tokens 1024:1032) attention -------------------

        def qc2_scores(pair, h_in_pair):
            qoff, qsz = QC2
            h = 2 * pair + h_in_pair
            p0 = 64 * h_in_pair
            sc = ps_pr.tile([128, 512], F32, tag="pr", name="ps_pr")
            for kt in range(9):
                koff, ksz = KT[kt]
                nc.tensor.matmul(
                    sc[0:ksz, kt * 8:kt * 8 + qsz],
                    qkT[6 + pair][p0:p0 + 64, koff:koff + ksz],
                    qkT[pair][p0:p0 + 64, qoff:qoff + qsz],
                    start=True, stop=True,
                )
            a = p_at.tile([128, 1024], ATTN, tag="attnT", name="attnT", bufs=6)
            nc.scalar.activation(
                a[0:128, 0:64], sc[0:128, 0:64],
                mybir.ActivationFunctionType.Exp, scale=SCALE,
            )
            nc.scalar.activation(
                a[0:8, 64:72], sc[0:8, 64:72],
                mybir.ActivationFunctionType.Exp, scale=SCALE,
            )
            return (pair, h_in_pair, h, a)

        def qc2_finish(state):
            qoff, qsz = QC2
            pair, h_in_pair, h, a = state
            av = ps_av.tile([128, 512], F32, tag="av", name="ps_av")
            for kt in range(9):
                koff, ksz = KT[kt]
                nc.tensor.matmul(
                    av[0:128, 0:qsz],
                    v_ext[kt][0:ksz, h * VW:h * VW + 128],
                    a[0:ksz, kt * 8:kt * 8 + qsz],
                    start=(kt == 0), stop=(kt == 8),
                    skip_group_check=True,
                )
            normalize(av, pair, h_in_pair, qoff, qsz)

        def qc2_unit(pair, h_in_pair):
            def go():
                qc2_finish(qc2_scores(pair, h_in_pair))
            return go

        # ---------- schedule ---------------------------------------------------

        with nc.named_scope("pass0"):
            cs0 = make_chunk_state()
            for u in qk_pair_units(0):
                u()
            d0, b0 = {}, {}
            for p in range(6):
                fillers = []
                if p == 0:
                    fillers += v_units()
                if p < 5:
                    fillers += qk_pair_units(p + 1)
                attn_block(p, QC[0][0], QC[0][1], fillers,
                           fill_per_kt=(3 if p == 0 else 1), cs=cs0)
                if p > 1:
                    pair_norm_c(cs0, p - 2, b0[p - 2], QC[0][0], QC[0][1])
                if p > 0:
                    b0[p - 1] = pair_norm_b(cs0, p - 1, d0[p - 1], QC[0][1])
                d0[p] = pair_norm_a(cs0, p)

        with nc.named_scope("pass1"):
            cs1 = make_chunk_state()
            # e(c0) fillers start at block 2: chunk-0's last normalize
            # multiplies (pair 5) are emitted after pass1 block 1
            p1_fillers = {
                0: [],
                1: [],
                2: [e_unit(0, *QC[0]), e_unit(1, *QC[0]),
                    qc2_unit(0, 0), qc2_unit(0, 1)],
                3: [e_unit(2, *QC[0]), qc2_unit(1, 0), qc2_unit(1, 1)],
                4: [e_unit(3, *QC[0]), qc2_unit(2, 0), qc2_unit(2, 1)],
                5: [e_unit(4, *QC[0]), e_unit(5, *QC[0]),
                    qc2_unit(3, 0), qc2_unit(3, 1)],
            }
            d1, b1 = {}, {}
            for p in range(6):
                attn_block(p, QC[1][0], QC[1][1], p1_fillers[p],
                           fill_per_kt=1, cs=cs1, warm=True)
                d1[p] = pair_norm_a(cs1, p)
                if p > 0:
                    b1[p - 1] = pair_norm_b(cs1, p - 1, d1[p - 1], QC[1][1])
                if p == 0:
                    b0[5] = pair_norm_b(cs0, 5, d0[5], QC[0][1])
                    pair_norm_c(cs0, 4, b0[4], QC[0][0], QC[0][1])
                elif p == 1:
                    pair_norm_c(cs0, 5, b0[5], QC[0][0], QC[0][1])
                else:
                    pair_norm_c(cs1, p - 2, b1[p - 2], QC[1][0], QC[1][1])
                if p == 5:
                    b1[5] = pair_norm_b(cs1, 5, d1[5], QC[1][1])

        with nc.named_scope("tail"):
            # chunk-1 pair 4/5 broadcast chains were emitted during pass1, so
            # only their multiplies remain; qc2 score work covers them, then
            # e(c1) projections weave with the qc2 finishes
            pend2 = []
            pend2.append(qc2_scores(4, 0))
            pend2.append(qc2_scores(4, 1))
            pair_norm_c(cs1, 4, b1[4], QC[1][0], QC[1][1])
            qc2_finish(pend2.pop(0))
            pend2.append(qc2_scores(5, 0))
            pair_norm_c(cs1, 5, b1[5], QC[1][0], QC[1][1])
            qc2_finish(pend2.pop(0))
            for i, ph in enumerate(((5, 1),)):
                e_unit(i, *QC[1])()
                pend2.append(qc2_scores(*ph))
                qc2_finish(pend2.pop(0))
            e_unit(1, *QC[1])()
            qc2_finish(pend2.pop(0))
            for ot in range(2, CT):
                e_unit(ot, *QC[1])()
            for ot in range(CT):
                e_unit(ot, QC2[0], QC2[1])()

    nc.compile()
    return nc


def _memset(nc, AT, ap, one):
    if AT == BF16:
        nc.vector.memset(ap.bitcast(mybir.dt.uint16), 0x3F80 if one else 0)
    elif AT == F16:
        nc.vector.memset(ap.bitcast(mybir.dt.uint16), 0x3C00 if one else 0)
    else:
        nc.vector.memset(ap.bitcast(mybir.dt.uint32), 0x3F800000 if one else 0)


_NC_CACHE = {}
_MODE = "fp16"


def prep_in_maps(x, w_qkv, w_proj, b_proj, mode=None):
    mode = mode or _MODE
    x = np.asarray(x, np.float32)
    w_qkv = np.asarray(w_qkv, np.float32)
    w_proj = np.asarray(w_proj, np.float32)
    b_proj = np.asarray(b_proj, np.float32)
    B = x.shape[0]
    assert x.shape == (8, NTOK, C), x.shape

    mt = np.float16
    # wqkT columns grouped by head-pair: [q_p | k_p] blocks of 256
    wqkT = np.zeros((C, 2 * C), mt)
    for p in range(6):
        wqkT[:, 256 * p:256 * p + 128] = w_qkv[128 * p:128 * (p + 1)].T
        wqkT[:, 256 * p + 128:256 * p + 256] = \
            w_qkv[C + 128 * p:C + 128 * (p + 1)].T
    wvT = np.ascontiguousarray(w_qkv[2 * C:].T.astype(mt))
    wpT = np.ascontiguousarray(w_proj.T.astype(mt))
    bp = np.ascontiguousarray(b_proj.reshape(C, 1))
    in_maps = []
    for b in range(B):
        xT = np.zeros((C, T), mt)
        xT[:, :NTOK] = x[b].T.astype(mt)
        in_maps.append({"xT": xT, "wqkT": wqkT, "wvT": wvT, "wpT": wpT,
                        "bp": bp})
    return in_maps


def kernel(x, w_qkv, w_proj, b_proj):
    B = np.asarray(x).shape[0]
    in_maps = prep_in_maps(x, w_qkv, w_proj, b_proj, _MODE)

    if _MODE not in _NC_CACHE:
        _NC_CACHE[_MODE] = build(matmul_dtype=_MODE)
    nc = _NC_CACHE[_MODE]
    from concourse import bass_utils
    res = bass_utils.run_bass_kernel_spmd(nc, in_maps, core_ids=list(range(B)),
                                          trace=False)
    y = np.stack([res.results[b]["yT"][:, :NTOK].T for b in range(B)])
    return np.ascontiguousarray(y.astype(np.float32))


# revision 21
# speedup vs baseline: 1.0064x; 1.0064x over previous
"""Trainium2 Bass kernel: batched multi-head self-attention (nn_Attention).

y = softmax(q k^T / sqrt(64)) v, projected; x (8, 1025, 768), 12 heads x 64.

Strategy: batch-parallel across the 8 NeuronCores (one batch element per
core, no collectives). Per core, everything is feature-major (transposed).

Software-pipelined single pass: the qk projection for head-pair p+1, the
v projection, and the output projection are woven as filler matmuls between
the attention steps of pair p, so the scalar engine (softmax exp) starts
early and never stalls behind a projection phase. Score matmuls for the two
heads of a pair run CONCURRENTLY in the PE array (64x128 row tiling: h0 in
rows 0-63, h1 in rows 64-127, distinct PSUM banks). Scores for both heads
of one key tile land in one [128, 1024] PSUM tile so each key tile needs a
single 1024-wide exp. The 8-key tail tile puts h0 at cols 0:512 and h1 at
512:1024 (rows 0:8 both) so one exp covers it and no v-row duplication is
needed. Softmax normalization runs per-pair right after each pair's block,
entirely off the PE queue (DVE reciprocal, Pool-engine partition
broadcast, DVE multiplies) so it never stalls matmuls; the kernel tail
only waits on pair 5's short chain before the last output projections. Inputs stream in as
full-width tiles (2KB rows) interleaved so the first projection starts as
early as the DMA allows; wqk is column-grouped by head-pair on the host so
the first pair's weights are one contiguous slice.

Operands fp16 (inputs/weights/q/k/v), exp'd weights bf16, accumulation
fp32 in PSUM.
"""
import sys

try:
    import concourse.bass  # noqa: F401
except ImportError:
    sys.path.insert(0, "/opt/trn_rl_repo")

import numpy as np

from contextlib import ExitStack

import concourse.bass as bass
import concourse.tile as tile
from concourse import bacc, mybir

F32 = mybir.dt.float32
BF16 = mybir.dt.bfloat16
F16 = mybir.dt.float16

C = 768
H = 12
D = 64
NTOK = 1025
T = 1032
CT = C // 128
SCALE = D ** -0.5

KT = [(i * 128, 128) for i in range(8)] + [(1024, 8)]
QC = [(0, 512), (512, 512)]
QC2 = (1024, 8)
VC = [(0, 512), (512, 256)]
VW = 65


def build(matmul_dtype="fp16"):
    MT = AT = F16
    ATTN = BF16
    nc = bacc.Bacc("TRN2", target_bir_lowering=False, debug=False, num_devices=8)

    xT_d = nc.dram_tensor("xT", [C, T], MT, kind="ExternalInput")
    # wqkT columns grouped by pair: [q_p0 | k_p0 | q_p1 | k_p1 | ...] (256 each)
    wqkT_d = nc.dram_tensor("wqkT", [C, 2 * C], MT, kind="ExternalInput")
    wvT_d = nc.dram_tensor("wvT", [C, C], MT, kind="ExternalInput")
    wpT_d = nc.dram_tensor("wpT", [C, C], MT, kind="ExternalInput")
    bp_d = nc.dram_tensor("bp", [C, 1], F32, kind="ExternalInput")
    yT_d = nc.dram_tensor("yT", [C, T], F16, kind="ExternalOutput")

    with tile.TileContext(nc) as tc, ExitStack() as ctx:
        p_x = ctx.enter_context(tc.tile_pool(name="x", bufs=1))
        p_w = ctx.enter_context(tc.tile_pool(name="w", bufs=1))
        p_qk = ctx.enter_context(tc.tile_pool(name="qk", bufs=1))
        p_v = ctx.enter_context(tc.tile_pool(name="v", bufs=1))
        p_ao = ctx.enter_context(tc.tile_pool(name="ao", bufs=1))
        p_at = ctx.enter_context(tc.tile_pool(name="at", bufs=1))
        p_sm = ctx.enter_context(tc.tile_pool(name="sm", bufs=1))
        p_stage = ctx.enter_context(tc.tile_pool(name="stage", bufs=1))
        ps_sc = ctx.enter_context(tc.tile_pool(name="psc", bufs=2, space="PSUM"))
        ps_av = ctx.enter_context(tc.tile_pool(name="psav", bufs=2, space="PSUM"))
        ps_pr = ctx.enter_context(tc.tile_pool(name="pspr", bufs=2, space="PSUM"))

        xT = [p_x.tile([128, T], MT, tag=f"x{i}", name=f"x{i}") for i in range(CT)]
        wqk = [p_w.tile([128, 2 * C], MT, tag=f"wqk{i}", name=f"wqk{i}")
               for i in range(CT)]
        wv = [p_w.tile([128, C], MT, tag=f"wv{i}", name=f"wv{i}") for i in range(CT)]
        wp = [p_w.tile([128, C], MT, tag=f"wp{i}", name=f"wp{i}") for i in range(CT)]
        bp_sb = [p_w.tile([128, 1], F32, tag=f"bp{i}", name=f"bp{i}")
                 for i in range(CT)]
        qkT = [p_qk.tile([128, T], AT, tag=f"qkT{i}", name=f"qkT{i}")
               for i in range(12)]
        v_ext = [p_v.tile([128, H * VW + 63], AT, tag=f"v{i}", name=f"v{i}")
                 for i in range(9)]
        aoT = [p_ao.tile([128, T], MT, tag=f"ao{i}", name=f"ao{i}")
               for i in range(CT)]

        # full-width tiles (2KB rows — small-packet DMA runs at a fraction of
        # peak), interleaved so pair 0's weights arrive alongside x
        for c in range(CT):
            nc.sync.dma_start(xT[c][:], xT_d.ap()[c * 128:(c + 1) * 128, :])
            nc.sync.dma_start(wqk[c][:, 0:256],
                              wqkT_d.ap()[c * 128:(c + 1) * 128, 0:256])
        for c in range(CT):
            nc.sync.dma_start(wv[c][:], wvT_d.ap()[c * 128:(c + 1) * 128, :])
        for c in range(CT):
            nc.sync.dma_start(wqk[c][:, 256:1536],
                              wqkT_d.ap()[c * 128:(c + 1) * 128, 256:1536])
        for c in range(CT):
            nc.sync.dma_start(wp[c][:], wpT_d.ap()[c * 128:(c + 1) * 128, :])
            nc.sync.dma_start(bp_sb[c][:], bp_d.ap()[c * 128:(c + 1) * 128, :])

        def wqk_col(p, is_k):
            return 256 * p + 128 * is_k

        # ---------- filler units (each emits ~one PSUM chunk of proj work) ----

        def qk_unit(p, is_k, qoff, qsz):
            ot = 6 * is_k + p

            def go():
                ps = ps_pr.tile([128, 512], F32, tag="pr", name="ps_pr")
                for c in range(CT):
                    nc.tensor.matmul(
                        ps[:, :qsz],
                        wqk[c][:, wqk_col(p, is_k):wqk_col(p, is_k) + 128],
                        xT[c][:, qoff:qoff + qsz],
                        start=(c == 0), stop=(c == CT - 1),
                    )
                nc.vector.tensor_copy(qkT[ot][:, qoff:qoff + qsz], ps[:, :qsz])
            return go

        def qk_pair_units(p):
            order = [(p, 0, QC[0]), (p, 1, QC[0]), (p, 1, QC[1]),
                     (p, 1, QC2), (p, 0, QC[1]), (p, 0, QC2)]
            return [qk_unit(pp, k, qoff, qsz) for pp, k, (qoff, qsz) in order]

        def v_unit(nt, voff, vsz):
            noff, nsz = KT[nt]

            def go():
                ps = ps_pr.tile([128, 512], F32, tag="pr", name="ps_pr")
                for c in range(CT):
                    nc.tensor.matmul(
                        ps[:nsz, :vsz],
                        xT[c][:, noff:noff + nsz],
                        wv[c][:, voff:voff + vsz],
                        start=(c == 0), stop=(c == CT - 1),
                    )
                nh = vsz // D
                h0 = voff // D
                dst = (
                    v_ext[nt][0:nsz, h0 * VW:(h0 + nh) * VW]
                    .rearrange("p (hh w) -> p hh w", w=VW)[:, :, 0:D]
                )
                src = ps[0:nsz, 0:vsz].rearrange("p (hh w) -> p hh w", w=D)
                nc.vector.tensor_copy(dst, src)
                if voff + vsz == C:
                    # ones column (valid tokens only) + zeroed pad/tail
                    if nt < 8:
                        ones_col = (
                            v_ext[nt][0:nsz, 0:H * VW]
                            .rearrange("p (hh w) -> p hh w", w=VW)[:, :, D:VW]
                        )
                        _memset(nc, AT, ones_col, one=True)
                    else:
                        pad_col = (
                            v_ext[nt][0:nsz, 0:H * VW]
                            .rearrange("p (hh w) -> p hh w", w=VW)[:, :, D:VW]
                        )
                        _memset(nc, AT, pad_col, one=False)
                        one_row = (
                            v_ext[nt][0:1, 0:H * VW]
                            .rearrange("p (hh w) -> p hh w", w=VW)[:, :, D:VW]
                        )
                        _memset(nc, AT, one_row, one=True)
                    _memset(nc, AT, v_ext[nt][:, H * VW:H * VW + 63], one=False)
            return go

        def v_units():
            return [v_unit(nt, voff, vsz) for nt in range(9) for (voff, vsz) in VC]

        def e_unit(ot, qoff, qsz):
            def go():
                ps = ps_pr.tile([128, 512], F32, tag="pr", name="ps_pr")
                for c in range(CT):
                    nc.tensor.matmul(
                        ps[:, :qsz],
                        wp[c][:, ot * 128:(ot + 1) * 128],
                        aoT[c][:, qoff:qoff + qsz],
                        start=(c == 0), stop=(c == CT - 1),
                    )
                st = p_stage.tile([128, 512], F16, tag="ystage", name="ystage",
                                  bufs=4)
                nc.vector.tensor_scalar_add(st[:, :qsz], ps[:, :qsz],
                                            bp_sb[ot][:, 0:1])
                nc.sync.dma_start(
                    yT_d.ap()[ot * 128:(ot + 1) * 128, qoff:qoff + qsz],
                    st[:, :qsz])
            return go

        # ---------- attention block for one head pair, one query chunk -------

        def normalize(av_ps, pair, h_in_pair, qoff, qsz):
            # single-head normalize (tail queries: recip of [1, 8] is cheap)
            p0 = 64 * h_in_pair
            av_sb = p_sm.tile([VW, 512], F32, tag="avsb", name="avsb", bufs=2)
            nc.vector.tensor_copy(av_sb[0:VW, 0:qsz], av_ps[0:VW, 0:qsz])
            rec = p_sm.tile([1, 512], F32, tag="rec", name="rec", bufs=2)
            nc.vector.reciprocal(rec[0:1, 0:qsz], av_sb[D:VW, 0:qsz])
            bc = p_sm.tile([64, 512], F32, tag="bc", name="bc", bufs=2)
            nc.gpsimd.partition_broadcast(bc[0:64, 0:qsz], rec[0:1, 0:qsz])
            nc.vector.tensor_mul(
                aoT[pair][p0:p0 + 64, qoff:qoff + qsz],
                av_sb[0:D, 0:qsz],
                bc[0:64, 0:qsz],
            )

        # big query chunks: stash all 12 heads' av into one wide tile so the
        # AV psum banks free quickly
        def make_chunk_state():
            return {"av": p_sm.tile([VW, 12 * 512], F32, tag="avall",
                                    name="avall", bufs=2)}

        def stash_av(cs, av_ps, h, qoff, qsz):
            nc.vector.tensor_copy(cs["av"][0:VW, h * 512:h * 512 + qsz],
                                  av_ps[0:VW, 0:qsz])

        # per-pair normalize, fully off the PE queue and cheap on the DVE.
        # DVE reciprocal costs ~7ns/elem PER LANE (lanes = partitions), so a
        # tiny SBUF->SBUF DMA first spreads the pair's 1024 denominators
        # across 64 partitions ([64,16] -> ~0.4us reciprocal), a second DMA
        # flattens the result back to one partition for the Pool-engine
        # partition-broadcast (which only accepts partition-0 sources), and
        # two DVE multiplies write aoT. The three stages are emitted a block
        # apart so every op is ready the moment its strict-FIFO engine queue
        # dequeues it.
        def pair_norm_a(cs, pair):
            den64 = p_sm.tile([64, 16], F32, tag="den64", name="den64", bufs=3)
            nc.sync.dma_start(
                den64[0:64, 0:16],
                cs["av"][D:VW, 2 * pair * 512:(2 * pair + 2) * 512])
            return den64

        def pair_norm_b(cs, pair, den64, qsz):
            rec64 = p_sm.tile([64, 16], F32, tag="rec64", name="rec64", bufs=3)
            nc.vector.reciprocal(rec64[0:64, 0:16], den64[0:64, 0:16])
            rec = p_sm.tile([1, 1024], F32, tag="recp", name="recp", bufs=3)
            nc.sync.dma_start(rec[0:1, 0:1024], rec64[0:64, 0:16])
            bcs = []
            for hi in range(2):
                bc = p_sm.tile([64, 512], F32, tag="bcp", name="bcp", bufs=6)
                nc.gpsimd.partition_broadcast(
                    bc[0:64, 0:qsz], rec[0:1, hi * 512:hi * 512 + qsz])
                bcs.append(bc)
            return bcs

        def pair_norm_c(cs, pair, bcs, qoff, qsz):
            for hi in range(2):
                h = 2 * pair + hi
                nc.vector.tensor_mul(
                    aoT[pair][64 * hi:64 * hi + 64, qoff:qoff + qsz],
                    cs["av"][0:D, h * 512:h * 512 + qsz],
                    bcs[hi][0:64, 0:qsz],
                )

        def attn_block(pair, qoff, qsz, fillers, fill_per_kt, cs, warm=False):
            h0, h1 = 2 * pair, 2 * pair + 1
            fill_iter = iter(fillers)

            def fill(n):
                for _ in range(n):
                    f = next(fill_iter, None)
                    if f is not None:
                        f()

            avs = {
                h0: ps_av.tile([128, 512], F32, tag="av", name="ps_av"),
                h1: ps_av.tile([128, 512], F32, tag="av", name="ps_av"),
            }
            pend = []  # (kt, at_tile) awaiting AV emission

            def scores_kt(kt, warm=False):
                koff, ksz = KT[kt]
                sc = ps_sc.tile([128, 1024], F32, tag="sc", name="ps_sc")
                if warm:
                    # full-array dummy matmul holds the PE clock governor at
                    # 8/8 through the half-array score runs (overwritten by
                    # the real scores below)
                    nc.tensor.matmul(
                        sc[0:128, 0:512],
                        qkT[0][0:128, 0:128],
                        qkT[0][0:128, 0:512],
                        start=True, stop=True,
                    )
                # the two heads' score matmuls run concurrently (row groups
                # 0/64) into one psum tile: h0 at cols 0:512, h1 at 512:1024
                nc.tensor.matmul(
                    sc[0:ksz, 0:qsz],
                    qkT[6 + pair][0:64, koff:koff + ksz],
                    qkT[pair][0:64, qoff:qoff + qsz],
                    start=True, stop=True,
                )
                nc.tensor.matmul(
                    sc[0:ksz, 512:512 + qsz],
                    qkT[6 + pair][64:128, koff:koff + ksz],
                    qkT[pair][64:128, qoff:qoff + qsz],
                    start=True, stop=True,
                )
                at = p_at.tile([128, 1024], ATTN, tag="attnT", name="attnT", bufs=6)
                nc.scalar.activation(
                    at[0:ksz, 0:1024], sc[0:ksz, 0:1024],
                    mybir.ActivationFunctionType.Exp, scale=SCALE,
                )
                return (kt, at)

            def drain_av():
                while pend:
                    pkt, pat = pend.pop(0)
                    _, pksz = KT[pkt]
                    for hi, h in enumerate((h0, h1)):
                        nc.tensor.matmul(
                            avs[h][0:128, 0:qsz],
                            v_ext[pkt][0:pksz, h * VW:h * VW + 128],
                            pat[0:pksz, hi * 512:hi * 512 + qsz],
                            start=(pkt == 0), stop=False,
                            skip_group_check=True,
                        )

            # 2-kt bursts keep the PE in 64-row tiling mode across 4 score
            # matmuls before switching to 128-row AV/filler matmuls; AV
            # emission lags one burst so its exp input is ready
            for kb in range(4):
                new = [scores_kt(2 * kb, warm=warm), scores_kt(2 * kb + 1)]
                fill(fill_per_kt)
                drain_av()
                pend.extend(new)
                fill(fill_per_kt)

            # tail key tile (8 keys, rows 0:8): h0 scores at cols 0:512
            # (bank 0), h1 at 512:1024 (bank 1) -> one [8, 1024] exp, and the
            # tail AV reads v_ext[8] rows 0:8 for both heads
            koff, ksz = KT[8]
            sc8 = ps_sc.tile([128, 1024], F32, tag="sc", name="ps_sc")
            nc.tensor.matmul(
                sc8[0:8, 0:qsz],
                qkT[6 + pair][0:64, koff:koff + ksz],
                qkT[pair][0:64, qoff:qoff + qsz],
                start=True, stop=True,
            )
            nc.tensor.matmul(
                sc8[0:8, 512:512 + qsz],
                qkT[6 + pair][64:128, koff:koff + ksz],
                qkT[pair][64:128, qoff:qoff + qsz],
                start=True, stop=True,
            )
            at8 = p_at.tile([128, 1024], ATTN, tag="attnT", name="attnT", bufs=6)
            nc.scalar.activation(
                at8[0:8, 0:1024], sc8[0:8, 0:1024],
                mybir.ActivationFunctionType.Exp, scale=SCALE,
            )
            # drain pending AV (kt 6/7), then the tail AV closes accumulation
            while pend:
                pkt, pat = pend.pop(0)
                _, pksz = KT[pkt]
                for hi, h in enumerate((h0, h1)):
                    nc.tensor.matmul(
                        avs[h][0:128, 0:qsz],
                        v_ext[pkt][0:pksz, h * VW:h * VW + 128],
                        pat[0:pksz, hi * 512:hi * 512 + qsz],
                        start=False, stop=False,
                        skip_group_check=True,
                    )
            for hi, h in enumerate((h0, h1)):
                nc.tensor.matmul(
                    avs[h][0:128, 0:qsz],
                    v_ext[8][0:8, h * VW:h * VW + 128],
                    at8[0:8, hi * 512:hi * 512 + qsz],
                    start=False, stop=True,
                    skip_group_check=True,
                )
            stash_av(cs, avs[h0], h0, qoff, qsz)
            stash_av(cs, avs[h1], h1, qoff, qsz)
            fill(99)  # drain leftovers

        # ---------- tail-query (tokens 1024:1032) attention -------------------

        def qc2_scores(pair, h_in_pair):
            qoff, qsz = QC2
            h = 2 * pair + h_in_pair
            p0 = 64 * h_in_pair
            sc = ps_pr.tile([128, 512], F32, tag="pr", name="ps_pr")
            for kt in range(9):
                koff, ksz = KT[kt]
                nc.tensor.matmul(
                    sc[0:ksz, kt * 8:kt * 8 + qsz],
                    qkT[6 + pair][p0:p0 + 64, koff:koff + ksz],
                    qkT[pair][p0:p0 + 64, qoff:qoff + qsz],
                    start=True, stop=True,
                )
            a = p_at.tile([128, 1024], ATTN, tag="attnT", name="attnT", bufs=6)
            nc.scalar.activation(
                a[0:128, 0:64], sc[0:128, 0:64],
                mybir.ActivationFunctionType.Exp, scale=SCALE,
            )
            nc.scalar.activation(
                a[0:8, 64:72], sc[0:8, 64:72],
                mybir.ActivationFunctionType.Exp, scale=SCALE,
            )
            return (pair, h_in_pair, h, a)

        def qc2_finish(state):
            qoff, qsz = QC2
            pair, h_in_pair, h, a = state
            av = ps_av.tile([128, 512], F32, tag="av", name="ps_av")
            for kt in range(9):
                koff, ksz = KT[kt]
                nc.tensor.matmul(
                    av[0:128, 0:qsz],
                    v_ext[kt][0:ksz, h * VW:h * VW + 128],
                    a[0:ksz, kt * 8:kt * 8 + qsz],
                    start=(kt == 0), stop=(kt == 8),
                    skip_group_check=True,
                )
            normalize(av, pair, h_in_pair, qoff, qsz)

        def qc2_unit(pair, h_in_pair):
            def go():
                qc2_finish(qc2_scores(pair, h_in_pair))
            return go

        # ---------- schedule ---------------------------------------------------

        with nc.named_scope("pass0"):
            cs0 = make_chunk_state()
            for u in qk_pair_units(0):
                u()
            d0, b0 = {}, {}
            for p in range(6):
                fillers = []
                if p == 0:
                    fillers += v_units()
                if p < 5:
                    fillers += qk_pair_units(p + 1)
                attn_block(p, QC[0][0], QC[0][1], fillers,
                           fill_per_kt=(3 if p == 0 else 1), cs=cs0)
                if p > 1:
                    pair_norm_c(cs0, p - 2, b0[p - 2], QC[0][0], QC[0][1])
                if p > 0:
                    b0[p - 1] = pair_norm_b(cs0, p - 1, d0[p - 1], QC[0][1])
                d0[p] = pair_norm_a(cs0, p)

        with nc.named_scope("pass1"):
            cs1 = make_chunk_state()
            # e(c0) fillers start at block 2: chunk-0's last normalize
            # multiplies (pair 5) are emitted after pass1 block 1
            p1_fillers = {
                0: [],
                1: [],
                2: [e_unit(0, *QC[0]), e_unit(1, *QC[0]),
                    qc2_unit(0, 0), qc2_unit(0, 1)],
                3: [e_unit(2, *QC[0]), qc2_unit(1, 0), qc2_unit(1, 1)],
                4: [e_unit(3, *QC[0]), qc2_unit(2, 0), qc2_unit(2, 1)],
                5: [e_unit(4, *QC[0]), e_unit(5, *QC[0]),
                    qc2_unit(3, 0), qc2_unit(3, 1)],
            }
            d1, b1 = {}, {}
            for p in range(6):
                attn_block(p, QC[1][0], QC[1][1], p1_fillers[p],
                           fill_per_kt=1, cs=cs1, warm=True)
                d1[p] = pair_norm_a(cs1, p)
                if p > 0:
                    b1[p - 1] = pair_norm_b(cs1, p - 1, d1[p - 1], QC[1][1])
                if p == 0:
                    b0[5] = pair_norm_b(cs0, 5, d0[5], QC[0][1])
                    pair_norm_c(cs0, 4, b0[4], QC[0][0], QC[0][1])
                elif p == 1:
                    pair_norm_c(cs0, 5, b0[5], QC[0][0], QC[0][1])
                else:
                    pair_norm_c(cs1, p - 2, b1[p - 2], QC[1][0], QC[1][1])
                if p == 5:
                    b1[5] = pair_norm_b(cs1, 5, d1[5], QC[1][1])

        with nc.named_scope("tail"):
            # chunk-1 pair 4/5 broadcast chains were emitted during pass1, so
            # only their multiplies remain; qc2 score work covers them, then
            # e(c1) projections weave with the qc2 finishes. Dummy full-array
            # warmers bridge the remaining wait on pair 4/5's normalize DMA
            # chains so the HAM clock gate never sees an idle window here
            # (the re-throttle cascade cost ~9us in the V10/V11 traces).
            def tail_warm(n):
                w = ps_sc.tile([128, 1024], F32, tag="sc", name="ps_sc")
                for _ in range(n):
                    nc.tensor.matmul(
                        w[0:128, 0:512],
                        qkT[0][0:128, 0:128],
                        qkT[0][0:128, 0:512],
                        start=True, stop=True,
                    )
            pend2 = []
            pend2.append(qc2_scores(4, 0))
            pend2.append(qc2_scores(4, 1))
            pair_norm_c(cs1, 4, b1[4], QC[1][0], QC[1][1])
            qc2_finish(pend2.pop(0))
            pend2.append(qc2_scores(5, 0))
            pair_norm_c(cs1, 5, b1[5], QC[1][0], QC[1][1])
            qc2_finish(pend2.pop(0))
            tail_warm(10)
            for i, ph in enumerate(((5, 1),)):
                e_unit(i, *QC[1])()
                pend2.append(qc2_scores(*ph))
                qc2_finish(pend2.pop(0))
            e_unit(1, *QC[1])()
            qc2_finish(pend2.pop(0))
            for ot in range(2, CT):
                e_unit(ot, *QC[1])()
            for ot in range(CT):
                e_unit(ot, QC2[0], QC2[1])()

    nc.compile()
    return nc


def _memset(nc, AT, ap, one):
    if AT == BF16:
        nc.vector.memset(ap.bitcast(mybir.dt.uint16), 0x3F80 if one else 0)
    elif AT == F16:
        nc.vector.memset(ap.bitcast(mybir.dt.uint16), 0x3C00 if one else 0)
    else:
        nc.vector.memset(ap.bitcast(mybir.dt.uint32), 0x3F800000 if one else 0)


_NC_CACHE = {}
_MODE = "fp16"


def prep_in_maps(x, w_qkv, w_proj, b_proj, mode=None):
    mode = mode or _MODE
    x = np.asarray(x, np.float32)
    w_qkv = np.asarray(w_qkv, np.float32)
    w_proj = np.asarray(w_proj, np.float32)
    b_proj = np.asarray(b_proj, np.float32)
    B = x.shape[0]
    assert x.shape == (8, NTOK, C), x.shape

    mt = np.float16
    # wqkT columns grouped by head-pair: [q_p | k_p] blocks of 256
    wqkT = np.zeros((C, 2 * C), mt)
    for p in range(6):
        wqkT[:, 256 * p:256 * p + 128] = w_qkv[128 * p:128 * (p + 1)].T
        wqkT[:, 256 * p + 128:256 * p + 256] = \
            w_qkv[C + 128 * p:C + 128 * (p + 1)].T
    wvT = np.ascontiguousarray(w_qkv[2 * C:].T.astype(mt))
    wpT = np.ascontiguousarray(w_proj.T.astype(mt))
    bp = np.ascontiguousarray(b_proj.reshape(C, 1))
    in_maps = []
    for b in range(B):
        xT = np.zeros((C, T), mt)
        xT[:, :NTOK] = x[b].T.astype(mt)
        in_maps.append({"xT": xT, "wqkT": wqkT, "wvT": wvT, "wpT": wpT,
                        "bp": bp})
    return in_maps


def kernel(x, w_qkv, w_proj, b_proj):
    B = np.asarray(x).shape[0]
    in_maps = prep_in_maps(x, w_qkv, w_proj, b_proj, _MODE)

    if _MODE not in _NC_CACHE:
        _NC_CACHE[_MODE] = build(matmul_dtype=_MODE)
    nc = _NC_CACHE[_MODE]
    from concourse import bass_utils
    res = bass_utils.run_bass_kernel_spmd(nc, in_maps, core_ids=list(range(B)),
                                          trace=False)
    y = np.stack([res.results[b]["yT"][:, :NTOK].T for b in range(B)])
    return np.ascontiguousarray(y.astype(np.float32))
